# revision 14
# baseline (speedup 1.0000x reference)
"""Trainium2 Bass kernel for nn_ClinicalEmbedding (EmbeddingBag-style ragged gather).

Semantics (matches reference.py):
  flat = codes.reshape(B, L); g = renorm(W[flat])  (max_norm=1.0)
  out[b, v] = 0                       for v <  V - nv[b]
            = g[b, v - (V-nv[b])]     for V-nv[b] <= v < V-1
            = sum_{j=nv-1}^{nv*C-1} g[b, j]   for v = V-1

Sharding: data-parallel over batch across 8 cores, W replicated.

Device strategy (v5): everything is fetched with large packed dma_gather
calls (int16 indices -> vocab split into 4x32768-row buckets; np.unique
gives sorted uniques so buckets are contiguous runs), round-robined over
4 SWDGE queues so descriptor generation runs on all Q7 core pairs in
parallel (~3x faster than one queue). The queue pattern must stay
lane-consistent with Tile's 8-lane DMASW semaphore rotation, so ALL
SWDGE DMAs are dma_gather pieces with queue_num = emission_index % 4.

Bag sums: for slot s,  sum_u CNT[u, s] * rsqrt(max(1, |W_u|^2)) * W_u
over per-core UNIQUE bag codes; one 128x32 @ 128xE fp32 matmul per
128-row chunk accumulates in PSUM. Singles (individual visit rows) are
deduped per core, gathered bucket-sorted, renormalized, and stored as a
contiguous block; the host unpermutes them into (b, v) slots (pad rows
stay zero by construction).
"""

import os

import numpy as np

import concourse.bacc as bacc
import concourse.bass as bass
import concourse.mybir as mybir
import concourse.tile as tile
from concourse.bass_utils import run_bass_kernel_spmd

P = 128          # SBUF partitions
N_CORES = 8
GCH = 8          # chunks per compute block (aligned to 1024-idx gather pieces)
BUCK = 1 << 15   # vocab rows per dma_gather bucket (int16 index range)
CH_G = 1024      # max idxs per dma_gather piece (64 descs/engine packet limit)
NQ_G = 4         # SWDGE queues, round-robined in emission order

LAST_RESULTS = None   # test harness reads profiling info from here


def _bucketize(uniq, bmax, NBUCK):
    """Split sorted unique ids into per-bucket segments padded to sizes bmax
    (each a multiple of 128). Returns (flat_local_ids, pos_of_uniq)."""
    NB = int(bmax.sum())
    flat = np.zeros(NB, np.int64)
    pos = np.full(len(uniq), -1, np.int64)
    offs = np.concatenate([[0], np.cumsum(bmax)]).astype(np.int64)
    for j in range(NBUCK):
        m = (uniq >= j * BUCK) & (uniq < (j + 1) * BUCK)
        seg = uniq[m]
        o = int(offs[j])
        flat[o : o + len(seg)] = seg - j * BUCK
        pos[np.where(m)[0]] = o + np.arange(len(seg))
    return flat, pos, offs


def _prepare(codes, nv, B, V, C, L, VOCAB, E):
    """Host-side index/count construction. Returns static structure + per-core data."""
    B_LOC = B // N_CORES
    NBUCK = (VOCAB + BUCK - 1) // BUCK
    nbag = nv * (C - 1) + 1                    # bag length per patient

    # balanced LPT assignment: sort desc by bag length, give to least-loaded core
    order = np.argsort(-nbag, kind="stable")
    loads = np.zeros(N_CORES, dtype=np.int64)
    counts = np.zeros(N_CORES, dtype=np.int64)
    assign = np.zeros((N_CORES, B_LOC), dtype=np.int64)
    for b in order:
        k = min((kk for kk in range(N_CORES) if counts[kk] < B_LOC),
                key=lambda kk: loads[kk])
        assign[k, counts[k]] = b
        counts[k] += 1
        loads[k] += nbag[b]

    # ---- per-core unique code sets ----
    bag_u, bag_cnt, bag_bs = [], [], []
    sg_u, sg_entries, sg_bs = [], [], []
    for k in range(N_CORES):
        bvals, bslots = [], []
        svals, s_sv = [], []
        for s in range(B_LOC):
            b = assign[k, s]
            n = int(nv[b])
            bvals.append(codes[b, n - 1 : n * C])
            bslots.append(np.full(n * (C - 1) + 1, s, dtype=np.int64))
            if n > 1:
                svals.append(codes[b, 0 : n - 1])
                vv = np.arange(V - n, V - 1)       # output visit rows
                s_sv.append(np.stack([np.full(n - 1, s), vv], axis=1))
        bv = np.concatenate(bvals)
        bs = np.concatenate(bslots)
        uniq, inv = np.unique(bv, return_inverse=True)   # sorted -> bucket runs
        cnt = np.zeros((len(uniq), B_LOC), np.float32)
        np.add.at(cnt, (inv, bs), 1.0)
        bag_u.append(uniq)
        bag_cnt.append(cnt)
        bag_bs.append(np.bincount(uniq // BUCK, minlength=NBUCK))

        sv = np.concatenate(svals) if svals else np.zeros(0, np.int64)
        se = np.concatenate(s_sv) if s_sv else np.zeros((0, 2), np.int64)
        su, sinv = np.unique(sv, return_inverse=True)
        sg_u.append(su)
        sg_entries.append((se, sinv))              # (s, v) rows + unique idx
        sg_bs.append(np.bincount(su // BUCK, minlength=NBUCK))

    r128 = lambda x: -(-x // P) * P
    SSb = np.array([r128(max(bs[j] for bs in sg_bs)) for j in range(NBUCK)],
                   dtype=np.int64)
    SB = np.array([r128(max(bs[j] for bs in bag_bs)) for j in range(NBUCK)],
                  dtype=np.int64)
    NS, NB = int(SSb.sum()), int(SB.sum())
    T_S, T_BAG = NS // P, NB // P
    T_ALL = T_S + T_BAG

    idx16 = np.zeros((N_CORES, P, (NS + NB) // 16), np.int16)
    CNT = np.zeros((N_CORES, P, T_BAG * B_LOC), np.float32)
    # host-side unpermute info: full[bvec, vvec] = singles_block[pvec, cvec]
    unperm = []

    for k in range(N_CORES):
        fsg, spos, _ = _bucketize(sg_u[k], SSb, NBUCK)
        fbag, _, _ = _bucketize(bag_u[k], SB, NBUCK)
        flat = np.concatenate([fsg, fbag])
        wrap = flat.astype(np.int16).reshape((NS + NB) // 16, 16).T
        idx16[k] = np.tile(wrap, (8, 1))

        cp = np.zeros((NB, B_LOC), np.float32)
        _, bpos, _ = _bucketize(bag_u[k], SB, NBUCK)
        cp[bpos] = bag_cnt[k]
        CNT[k] = cp.reshape(T_BAG, P, B_LOC).transpose(1, 0, 2).reshape(
            P, T_BAG * B_LOC
        )

        se, sinv = sg_entries[k]
        i_flat = spos[sinv]                        # flat gather position
        unperm.append((
            assign[k][se[:, 0]],                   # patient ids
            se[:, 1],                              # visit rows
            i_flat % P,                            # partition
            i_flat // P,                           # chunk
        ))

    # gather pieces: (global chunk offset, num idxs, in-bucket row base)
    pieces = []
    goff = 0
    for (sizes) in (SSb, SB):
        for j in range(NBUCK):
            nj = int(sizes[j])
            for o in range(0, nj, CH_G):
                pc = min(CH_G, nj - o)
                pieces.append((goff + o, pc, j))
            goff += nj

    return dict(
        B_LOC=B_LOC, T_S=T_S, T_BAG=T_BAG, T_ALL=T_ALL, NBUCK=NBUCK,
        pieces=pieces, assign=assign, idx16=idx16, CNT=CNT, unperm=unperm,
    )


def _build(prep, V, C, VOCAB, E):
    """Emit the Bass/Tile program (shared across all 8 cores)."""
    B_LOC, T_S, T_BAG, T_ALL = (
        prep["B_LOC"], prep["T_S"], prep["T_BAG"], prep["T_ALL"]
    )
    pieces = prep["pieces"]
    f32 = mybir.dt.float32
    i16 = mybir.dt.int16
    OUT_ROWS = B_LOC + P * T_S

    nc = bacc.Bacc("TRN2", num_devices=N_CORES, debug=False,
                   num_swdge_queues=NQ_G)
    W_d = nc.dram_tensor("W", [VOCAB, E], f32, kind="ExternalInput")
    idx16_d = nc.dram_tensor("idx16", [P, (T_ALL * P) // 16], i16,
                             kind="ExternalInput")
    cnt_d = nc.dram_tensor("CNT", [P, T_BAG * B_LOC], f32, kind="ExternalInput")
    out_d = nc.dram_tensor("out", [OUT_ROWS, E], f32, kind="ExternalOutput")

    # compute blocks: singles first (their data arrives first), then bag
    blocks = [(0, T_S)]
    c = T_S
    while c < T_ALL:
        blocks.append((c, min(c + GCH, T_ALL)))
        c = min(c + GCH, T_ALL)

    with tile.TileContext(nc) as tc:
        with (
            tc.tile_pool(name="const", bufs=1) as cpool,
            tc.tile_pool(name="g", bufs=1) as gpool,
            tc.tile_pool(name="sq", bufs=2) as sqpool,
            tc.tile_pool(name="sm", bufs=2) as smpool,
            tc.tile_pool(name="ps", bufs=1, space="PSUM") as pspool,
        ):
            idx16_t = cpool.tile_from(idx16_d[:])

            g = gpool.tile([P, T_ALL * E], f32, tag="g", bufs=1)
            n_t = smpool.tile([P, T_ALL], f32, tag="n", bufs=1)
            rn = smpool.tile([P, T_ALL], f32, tag="rn", bufs=1)
            M = gpool.tile([P, T_BAG * B_LOC], f32, tag="M", bufs=1)
            # bag sums accumulate transposed: psumT[e, s] (fewer streamed
            # matmul columns: lhsT=g streams only B_LOC=32 rhs columns)
            psumT = pspool.tile([P, B_LOC], f32)

            # zero bias tile written by DVE so ACT waits only on DVE
            zbias = smpool.tile([P, 1], f32, tag="zbias", bufs=1)
            nc.vector.memset(zbias[:], 0.0)

            # ---- gathers: packed dma_gather pieces, RR over SWDGE queues.
            # queue_num must equal emission_index % NQ_G so Tile's 8-lane
            # DMASW sem rotation stays queue-consistent per lane.
            for i, (o, pc, j) in enumerate(pieces):
                nrows = min(VOCAB, (j + 1) * BUCK) - j * BUCK
                nc.gpsimd.dma_gather(
                    out_ap=g[:, (o // P) * E : ((o + pc) // P) * E]
                    .rearrange("p (c e) -> p c e", e=E),
                    in_ap=W_d[j * BUCK : j * BUCK + nrows],
                    idxs_ap=idx16_t[:, o // 16 : (o + pc) // 16],
                    num_idxs=pc,
                    num_idxs_reg=pc,
                    elem_size=E,
                    single_packet=True,
                    queue_num=i % NQ_G,
                )

            # CNT is first read by the M-build, well after gathers start;
            # loading it after the gather issue keeps the idx16 load (which
            # gates the first gather) alone on the sync queue at t=0.
            cnt_t = cpool.tile_from(cnt_d[:])

            # ---- per-block compute ----
            for (c0, c1) in blocks:
                w = c1 - c0
                sq = sqpool.tile([P, max(GCH, T_S) * E], f32, tag="sq")
                nc.scalar.activation(
                    sq[:, : w * E], g[:, c0 * E : c1 * E],
                    mybir.ActivationFunctionType.Square, bias=zbias[:],
                )
                nc.vector.tensor_reduce(
                    n_t[:, c0:c1],
                    sq[:, : w * E].rearrange("p (c e) -> p c e", e=E),
                    axis=mybir.AxisListType.X, op=mybir.AluOpType.add,
                )
                nc.vector.tensor_scalar_max(n_t[:, c0:c1], n_t[:, c0:c1], 1.0)
                nc.scalar.activation(
                    rn[:, c0:c1], n_t[:, c0:c1],
                    mybir.ActivationFunctionType.Sqrt, bias=zbias[:],
                )
                nc.vector.reciprocal(rn[:, c0:c1], rn[:, c0:c1])

                if c0 == 0:
                    # singles: renormalize in place, store contiguous block
                    nc.vector.tensor_tensor(
                        out=g[:, : T_S * E].rearrange("p (c e) -> p c e", e=E),
                        in0=g[:, : T_S * E].rearrange("p (c e) -> p c e", e=E),
                        in1=rn[:, 0:T_S].to_broadcast([P, T_S, E]),
                        op=mybir.AluOpType.mult,
                    )
                    nc.sync.dma_start(
                        out=out_d[B_LOC:].rearrange("(p c) e -> p c e", c=T_S),
                        in_=g[:, : T_S * E].rearrange("p (c e) -> p c e", e=E),
                    )
                else:
                    t0, t1 = c0 - T_S, c1 - T_S
                    nc.vector.tensor_tensor(
                        out=M[:, t0 * B_LOC : t1 * B_LOC].rearrange(
                            "p (c s) -> p c s", s=B_LOC
                        ),
                        in0=cnt_t[:, t0 * B_LOC : t1 * B_LOC].rearrange(
                            "p (c s) -> p c s", s=B_LOC
                        ),
                        in1=rn[:, c0:c1].to_broadcast([P, w, B_LOC]),
                        op=mybir.AluOpType.mult,
                    )
                    for t in range(t0, t1):
                        nc.tensor.matmul(
                            out=psumT[:, :],
                            lhsT=g[:, (T_S + t) * E : (T_S + t + 1) * E],
                            rhs=M[:, t * B_LOC : (t + 1) * B_LOC],
                            start=(t == 0),
                            stop=(t == T_BAG - 1),
                        )

            # psumT [E, B_LOC] -> outS [B_LOC, E] via 4 DVE 32x32 transposes
            sbT = smpool.tile([P, B_LOC], f32, tag="sbT", bufs=1)
            nc.vector.tensor_copy(sbT[:], psumT[:])
            outS = smpool.tile([B_LOC, E], f32, tag="outS", bufs=1)
            for q in range(P // 32):
                nc.vector.transpose(
                    out=outS[:, q * 32 : (q + 1) * 32],
                    in_=sbT[q * 32 : (q + 1) * 32, :],
                )
            nc.sync.dma_start(out=out_d[:B_LOC], in_=outS[:])

    nc.compile()
    return nc


def kernel(**inputs) -> np.ndarray:
    global LAST_RESULTS
    W = np.ascontiguousarray(np.asarray(inputs["W"], dtype=np.float32))
    codes_in = np.asarray(inputs["codes"])
    nv = np.asarray(inputs["n_visits"]).astype(np.int64)

    B, V, C = codes_in.shape
    VOCAB, E = W.shape
    L = V * C
    codes = np.ascontiguousarray(codes_in.reshape(B, L).astype(np.int32))

    prep = _prepare(codes, nv, B, V, C, L, VOCAB, E)
    nc = _build(prep, V, C, VOCAB, E)

    in_maps = [
        {"W": W, "idx16": prep["idx16"][k], "CNT": prep["CNT"][k]}
        for k in range(N_CORES)
    ]
    trace = bool(int(os.environ.get("KERNEL_TRACE", "0")))
    res = run_bass_kernel_spmd(
        nc, in_maps, core_ids=list(range(N_CORES)), trace=trace
    )
    LAST_RESULTS = res

    B_LOC, T_S = prep["B_LOC"], prep["T_S"]
    assign = prep["assign"]
    full = np.zeros((B, V, E), np.float32)
    for k in range(N_CORES):
        o = res.results[k]["out"]
        full[assign[k], V - 1] = o[:B_LOC]
        sing = o[B_LOC:].reshape(P, T_S, E)
        bvec, vvec, pvec, cvec = prep["unperm"][k]
        full[bvec, vvec] = sing[pvec, cvec]
    return full


# revision 17
# speedup vs baseline: 1.4653x; 1.4653x over previous
"""Trainium2 Bass kernel for nn_ClinicalEmbedding (EmbeddingBag-style ragged gather).

Semantics (matches reference.py):
  flat = codes.reshape(B, L); g = renorm(W[flat])  (max_norm=1.0)
  out[b, v] = 0                       for v <  V - nv[b]
            = g[b, v - (V-nv[b])]     for V-nv[b] <= v < V-1
            = sum_{j=nv-1}^{nv*C-1} g[b, j]   for v = V-1

Sharding: data-parallel over batch across 8 cores, W replicated.

Device strategy (v5): everything is fetched with large packed dma_gather
calls (int16 indices -> vocab split into 4x32768-row buckets; np.unique
gives sorted uniques so buckets are contiguous runs), round-robined over
4 SWDGE queues so descriptor generation runs on all Q7 core pairs in
parallel (~3x faster than one queue). The queue pattern must stay
lane-consistent with Tile's 8-lane DMASW semaphore rotation, so ALL
SWDGE DMAs are dma_gather pieces with queue_num = emission_index % 4.

Bag sums: for slot s,  sum_u CNT[u, s] * rsqrt(max(1, |W_u|^2)) * W_u
over per-core UNIQUE bag codes; one 128x32 @ 128xE fp32 matmul per
128-row chunk accumulates in PSUM. Singles (individual visit rows) are
deduped per core, gathered bucket-sorted, renormalized, and stored as a
contiguous block; the host unpermutes them into (b, v) slots (pad rows
stay zero by construction).
"""

import os

import numpy as np

import concourse.bacc as bacc
import concourse.bass as bass
import concourse.mybir as mybir
import concourse.tile as tile
from concourse.bass_utils import run_bass_kernel_spmd

P = 128          # SBUF partitions
N_CORES = 8
GCH = 8          # chunks per compute block (aligned to 1024-idx gather pieces)
BUCK = 1 << 15   # vocab rows per dma_gather bucket (int16 index range)
CH_G = 1024      # max idxs per dma_gather piece (64 descs/engine packet limit)
NQ_G = 4         # SWDGE queues, round-robined in emission order

LAST_RESULTS = None   # test harness reads profiling info from here


def _bucketize(uniq, bmax, NBUCK):
    """Split sorted unique ids into per-bucket segments padded to sizes bmax
    (each a multiple of 128). Returns (flat_local_ids, pos_of_uniq)."""
    NB = int(bmax.sum())
    flat = np.zeros(NB, np.int64)
    pos = np.full(len(uniq), -1, np.int64)
    offs = np.concatenate([[0], np.cumsum(bmax)]).astype(np.int64)
    for j in range(NBUCK):
        m = (uniq >= j * BUCK) & (uniq < (j + 1) * BUCK)
        seg = uniq[m]
        o = int(offs[j])
        flat[o : o + len(seg)] = seg - j * BUCK
        pos[np.where(m)[0]] = o + np.arange(len(seg))
    return flat, pos, offs


def _prepare(codes, nv, B, V, C, L, VOCAB, E):
    """Host-side index/count construction. Returns static structure + per-core data."""
    B_LOC = B // N_CORES
    NBUCK = (VOCAB + BUCK - 1) // BUCK
    nbag = nv * (C - 1) + 1                    # bag length per patient

    # balanced LPT assignment: sort desc by bag length, give to least-loaded core
    order = np.argsort(-nbag, kind="stable")
    loads = np.zeros(N_CORES, dtype=np.int64)
    counts = np.zeros(N_CORES, dtype=np.int64)
    assign = np.zeros((N_CORES, B_LOC), dtype=np.int64)
    for b in order:
        k = min((kk for kk in range(N_CORES) if counts[kk] < B_LOC),
                key=lambda kk: loads[kk])
        assign[k, counts[k]] = b
        counts[k] += 1
        loads[k] += nbag[b]

    # ---- per-core unique code sets ----
    bag_u, bag_cnt, bag_bs = [], [], []
    sg_u, sg_entries, sg_bs = [], [], []
    for k in range(N_CORES):
        bvals, bslots = [], []
        svals, s_sv = [], []
        for s in range(B_LOC):
            b = assign[k, s]
            n = int(nv[b])
            bvals.append(codes[b, n - 1 : n * C])
            bslots.append(np.full(n * (C - 1) + 1, s, dtype=np.int64))
            if n > 1:
                svals.append(codes[b, 0 : n - 1])
                vv = np.arange(V - n, V - 1)       # output visit rows
                s_sv.append(np.stack([np.full(n - 1, s), vv], axis=1))
        bv = np.concatenate(bvals)
        bs = np.concatenate(bslots)
        uniq, inv = np.unique(bv, return_inverse=True)   # sorted -> bucket runs
        cnt = np.zeros((len(uniq), B_LOC), np.float32)
        np.add.at(cnt, (inv, bs), 1.0)
        bag_u.append(uniq)
        bag_cnt.append(cnt)
        bag_bs.append(np.bincount(uniq // BUCK, minlength=NBUCK))

        sv = np.concatenate(svals) if svals else np.zeros(0, np.int64)
        se = np.concatenate(s_sv) if s_sv else np.zeros((0, 2), np.int64)
        su, sinv = np.unique(sv, return_inverse=True)
        sg_u.append(su)
        sg_entries.append((se, sinv))              # (s, v) rows + unique idx
        sg_bs.append(np.bincount(su // BUCK, minlength=NBUCK))

    r128 = lambda x: -(-x // P) * P
    SSb = np.array([r128(max(bs[j] for bs in sg_bs)) for j in range(NBUCK)],
                   dtype=np.int64)
    SB = np.array([r128(max(bs[j] for bs in bag_bs)) for j in range(NBUCK)],
                  dtype=np.int64)
    NS, NB = int(SSb.sum()), int(SB.sum())
    T_S, T_BAG = NS // P, NB // P
    T_ALL = T_S + T_BAG

    idx16 = np.zeros((N_CORES, P, (NS + NB) // 16), np.int16)
    CNT = np.zeros((N_CORES, P, T_BAG * B_LOC), np.float32)
    # host-side unpermute info: full[bvec, vvec] = singles_block[pvec, cvec]
    unperm = []

    for k in range(N_CORES):
        fsg, spos, _ = _bucketize(sg_u[k], SSb, NBUCK)
        fbag, _, _ = _bucketize(bag_u[k], SB, NBUCK)
        flat = np.concatenate([fsg, fbag])
        wrap = flat.astype(np.int16).reshape((NS + NB) // 16, 16).T
        idx16[k] = np.tile(wrap, (8, 1))

        cp = np.zeros((NB, B_LOC), np.float32)
        _, bpos, _ = _bucketize(bag_u[k], SB, NBUCK)
        cp[bpos] = bag_cnt[k]
        CNT[k] = cp.reshape(T_BAG, P, B_LOC).transpose(1, 0, 2).reshape(
            P, T_BAG * B_LOC
        )

        se, sinv = sg_entries[k]
        i_flat = spos[sinv]                        # flat gather position
        unperm.append((
            assign[k][se[:, 0]],                   # patient ids
            se[:, 1],                              # visit rows
            i_flat % P,                            # partition
            i_flat // P,                           # chunk
        ))

    # gather pieces: (global chunk offset, num idxs, in-bucket row base)
    pieces = []
    goff = 0
    for (sizes) in (SSb, SB):
        for j in range(NBUCK):
            nj = int(sizes[j])
            for o in range(0, nj, CH_G):
                pc = min(CH_G, nj - o)
                pieces.append((goff + o, pc, j))
            goff += nj

    return dict(
        B_LOC=B_LOC, T_S=T_S, T_BAG=T_BAG, T_ALL=T_ALL, NBUCK=NBUCK,
        pieces=pieces, assign=assign, idx16=idx16, CNT=CNT, unperm=unperm,
    )


def _build(prep, V, C, VOCAB, E):
    """Emit the Bass/Tile program (shared across all 8 cores)."""
    B_LOC, T_S, T_BAG, T_ALL = (
        prep["B_LOC"], prep["T_S"], prep["T_BAG"], prep["T_ALL"]
    )
    pieces = prep["pieces"]
    f32 = mybir.dt.float32
    i16 = mybir.dt.int16
    OUT_ROWS = B_LOC + P * T_S

    nc = bacc.Bacc("TRN2", num_devices=N_CORES, debug=False,
                   num_swdge_queues=NQ_G)
    W_d = nc.dram_tensor("W", [VOCAB, E], f32, kind="ExternalInput")
    idx16_d = nc.dram_tensor("idx16", [P, (T_ALL * P) // 16], i16,
                             kind="ExternalInput")
    cnt_d = nc.dram_tensor("CNT", [P, T_BAG * B_LOC], f32, kind="ExternalInput")
    out_d = nc.dram_tensor("out", [OUT_ROWS, E], f32, kind="ExternalOutput")

    # compute blocks: singles first (their data arrives first), then bag
    blocks = [(0, T_S)]
    c = T_S
    while c < T_ALL:
        blocks.append((c, min(c + GCH, T_ALL)))
        c = min(c + GCH, T_ALL)

    with tile.TileContext(nc) as tc:
        with (
            tc.tile_pool(name="const", bufs=1) as cpool,
            tc.tile_pool(name="g", bufs=1) as gpool,
            tc.tile_pool(name="sq", bufs=2) as sqpool,
            tc.tile_pool(name="sm", bufs=2) as smpool,
            tc.tile_pool(name="ps", bufs=1, space="PSUM") as pspool,
        ):
            # split the idx16 load so the first gather pieces only wait on
            # their quarter of the index tile
            idx16_t = cpool.tile([P, (T_ALL * P) // 16], i16, tag="idx16",
                                 bufs=1)
            ncols = (T_ALL * P) // 16
            q4 = -(-ncols // 4)
            for q in range(0, ncols, q4):
                q1 = min(q + q4, ncols)
                nc.sync.dma_start(out=idx16_t[:, q:q1], in_=idx16_d[:, q:q1])

            g = gpool.tile([P, T_ALL * E], f32, tag="g", bufs=1)
            n_t = smpool.tile([P, T_ALL], f32, tag="n", bufs=1)
            rn = smpool.tile([P, T_ALL], f32, tag="rn", bufs=1)
            M = gpool.tile([P, T_BAG * B_LOC], f32, tag="M", bufs=1)
            psum = pspool.tile([B_LOC, E], f32)

            # zero bias tile written by DVE so ACT waits only on DVE
            zbias = smpool.tile([P, 1], f32, tag="zbias", bufs=1)
            nc.vector.memset(zbias[:], 0.0)

            # ---- gathers: packed dma_gather pieces, RR over SWDGE queues.
            # queue_num must equal emission_index % NQ_G so Tile's 8-lane
            # DMASW sem rotation stays queue-consistent per lane.
            for i, (o, pc, j) in enumerate(pieces):
                nrows = min(VOCAB, (j + 1) * BUCK) - j * BUCK
                nc.gpsimd.dma_gather(
                    out_ap=g[:, (o // P) * E : ((o + pc) // P) * E]
                    .rearrange("p (c e) -> p c e", e=E),
                    in_ap=W_d[j * BUCK : j * BUCK + nrows],
                    idxs_ap=idx16_t[:, o // 16 : (o + pc) // 16],
                    num_idxs=pc,
                    num_idxs_reg=pc,
                    elem_size=E,
                    single_packet=True,
                    queue_num=i % NQ_G,
                )

            # CNT is first read by the M-build, well after gathers start;
            # loading it after the gather issue keeps the idx16 load (which
            # gates the first gather) alone on the sync queue at t=0.
            cnt_t = cpool.tile_from(cnt_d[:])

            # ---- per-block compute ----
            for (c0, c1) in blocks:
                w = c1 - c0
                sq = sqpool.tile([P, max(GCH, T_S) * E], f32, tag="sq")
                nc.scalar.activation(
                    sq[:, : w * E], g[:, c0 * E : c1 * E],
                    mybir.ActivationFunctionType.Square, bias=zbias[:],
                )
                nc.vector.tensor_reduce(
                    n_t[:, c0:c1],
                    sq[:, : w * E].rearrange("p (c e) -> p c e", e=E),
                    axis=mybir.AxisListType.X, op=mybir.AluOpType.add,
                )
                nc.vector.tensor_scalar_max(n_t[:, c0:c1], n_t[:, c0:c1], 1.0)
                nc.scalar.activation(
                    rn[:, c0:c1], n_t[:, c0:c1],
                    mybir.ActivationFunctionType.Sqrt, bias=zbias[:],
                )
                nc.vector.reciprocal(rn[:, c0:c1], rn[:, c0:c1])

                if c0 == 0:
                    # singles: renormalize in place, store contiguous block
                    nc.vector.tensor_tensor(
                        out=g[:, : T_S * E].rearrange("p (c e) -> p c e", e=E),
                        in0=g[:, : T_S * E].rearrange("p (c e) -> p c e", e=E),
                        in1=rn[:, 0:T_S].to_broadcast([P, T_S, E]),
                        op=mybir.AluOpType.mult,
                    )
                    nc.sync.dma_start(
                        out=out_d[B_LOC:].rearrange("(p c) e -> p c e", c=T_S),
                        in_=g[:, : T_S * E].rearrange("p (c e) -> p c e", e=E),
                    )
                else:
                    t0, t1 = c0 - T_S, c1 - T_S
                    nc.vector.tensor_tensor(
                        out=M[:, t0 * B_LOC : t1 * B_LOC].rearrange(
                            "p (c s) -> p c s", s=B_LOC
                        ),
                        in0=cnt_t[:, t0 * B_LOC : t1 * B_LOC].rearrange(
                            "p (c s) -> p c s", s=B_LOC
                        ),
                        in1=rn[:, c0:c1].to_broadcast([P, w, B_LOC]),
                        op=mybir.AluOpType.mult,
                    )
                    for t in range(t0, t1):
                        nc.tensor.matmul(
                            out=psum[:, :],
                            lhsT=M[:, t * B_LOC : (t + 1) * B_LOC],
                            rhs=g[:, (T_S + t) * E : (T_S + t + 1) * E],
                            start=(t == 0),
                            stop=(t == T_BAG - 1),
                        )

            outS = smpool.tile([B_LOC, E], f32, tag="outS", bufs=1)
            nc.vector.tensor_copy(outS[:], psum[:])
            nc.sync.dma_start(out=out_d[:B_LOC], in_=outS[:])

    nc.compile()
    return nc


def kernel(**inputs) -> np.ndarray:
    global LAST_RESULTS
    W = np.ascontiguousarray(np.asarray(inputs["W"], dtype=np.float32))
    codes_in = np.asarray(inputs["codes"])
    nv = np.asarray(inputs["n_visits"]).astype(np.int64)

    B, V, C = codes_in.shape
    VOCAB, E = W.shape
    L = V * C
    codes = np.ascontiguousarray(codes_in.reshape(B, L).astype(np.int32))

    prep = _prepare(codes, nv, B, V, C, L, VOCAB, E)
    nc = _build(prep, V, C, VOCAB, E)

    in_maps = [
        {"W": W, "idx16": prep["idx16"][k], "CNT": prep["CNT"][k]}
        for k in range(N_CORES)
    ]
    trace = bool(int(os.environ.get("KERNEL_TRACE", "0")))
    res = run_bass_kernel_spmd(
        nc, in_maps, core_ids=list(range(N_CORES)), trace=trace
    )
    LAST_RESULTS = res

    B_LOC, T_S = prep["B_LOC"], prep["T_S"]
    assign = prep["assign"]
    full = np.zeros((B, V, E), np.float32)
    for k in range(N_CORES):
        o = res.results[k]["out"]
        full[assign[k], V - 1] = o[:B_LOC]
        sing = o[B_LOC:].reshape(P, T_S, E)
        bvec, vvec, pvec, cvec = prep["unperm"][k]
        full[bvec, vvec] = sing[pvec, cvec]
    return full
